# revision 37
# baseline (speedup 1.0000x reference)
"""Trainium2 Bass kernel for nn_CausalSelfAttention_42623255446168.

Contract: kernel(**inputs) takes FULL unsharded inputs (as produced by
setup_inputs()) and returns the FULL output [1, 2048, 1024] float32.

Sharding: tensor-parallel over the 16 query heads across 8 cores
(2 q-heads / core; each core uses exactly one GQA kv-head = core//2).
Each core computes a partial output projection [2048, 1024]; the host
sums the 8 partials (the "all-reduce" of the hint, done host-side).

v2 pipeline (vs the v1 baseline at ~281us):
  - ACT only ever uses the natural_log_exp table set (squares moved to
    DVE) -> zero ACT table swaps.
  - Scores st [128,2,512] in PSUM, one Exp per (kb, su-pair).
  - PV col-packed: the 4 differential-attention sub-units write 32-wide
    partition strips of ONE PSUM bank via tile_position=(0,32*su), all
    concurrent; softmax denominators via concurrent M=1 ones-matmuls
    into a second bank.  Normalize = one full-tile approx-reciprocal +
    4 concurrent K=1 broadcast matmuls + one DVE multiply (replaces the
    v1 per-su copy/recip/copy/matmul/mul/DMA chain).
  - Projection chunks interleaved with attention super-blocks in
    program order so PE/ACT/DVE overlap and the PE stays HAM-warm.
  - Input DMAs reordered (x chunk 0 + projection weights first).
"""

import functools
import math
import os
import sys

import numpy as np

sys.path.insert(0, "/opt/trn_rl_repo")

import concourse.bass as bass  # noqa: E402
import concourse.tile as tile  # noqa: E402
from concourse import bacc, mybir  # noqa: E402
from concourse.bass_utils import run_bass_kernel_spmd  # noqa: E402

S = 2048
DIM = 1024
H = 16
KVH = 4
HD = 64
HALF = 32
NCORES = 8
EPS = float(np.finfo(np.float32).eps)

F32 = mybir.dt.float32
F32R = mybir.dt.float32r
DEBUG = bool(int(os.environ.get("KBG_DEBUG", "0")))
PSUM = bass.MemorySpace.PSUM

QS = 4    # q-super blocks
QW = 512  # q-super width


def _r(ap):
    """Bitcast an f32 AP to f32r for full-rate PE matmuls."""
    return ap.bitcast(F32R)


def _build_kernel(tc, io):
    nc = tc.nc
    EXP = mybir.ActivationFunctionType.Exp
    LN = mybir.ActivationFunctionType.Ln

    with (
        tc.tile_pool(name="persist", bufs=1) as persist,
        tc.tile_pool(name="xpool", bufs=2) as xpool,
        tc.tile_pool(name="scr", bufs=2) as scr,
        tc.tile_pool(name="ptp", bufs=4) as ptp,
        tc.tile_pool(name="utp", bufs=2) as utp,
        tc.tile_pool(name="obp", bufs=2) as obp,
        tc.tile_pool(name="psOP", bufs=2, space=PSUM) as psOP,
        tc.tile_pool(name="psST", bufs=2, space=PSUM) as psST,
        tc.tile_pool(name="psAT", bufs=2, space=PSUM) as psAT,
    ):
        # ---------------- persistent tiles ----------------
        qRc = [persist.tile([128, QW], F32, name=f"qRc{i}") for i in range(QS)]
        kRc = [persist.tile([128, QW], F32, name=f"kRc{i}") for i in range(QS)]
        vac = [persist.tile([128, 4, 66], F32, name=f"vac{i}") for i in range(QS)]
        wq_s = persist.tile([128, 8, 128], F32R)
        wkd_s = persist.tile([128, 8, 128], F32R)
        wvg_s = persist.tile([128, 8, 128], F32R)
        wo_s = persist.tile([128, DIM], F32R)
        c1_s = persist.tile([128, S], F32)
        c2_s = persist.tile([128, S], F32)
        perm_s = persist.tile([128, 128], F32R)
        mg_s = persist.tile([2, 128], F32R)
        mones_s = persist.tile([1, 128], F32R)
        msq_s = persist.tile([128, 2], F32R)
        mok_s = persist.tile([64, 1], F32R)
        onesb_s = persist.tile([33, 32], F32R)
        ident_s = persist.tile([64, 64], F32)
        eps_t = persist.tile([128, 1], F32)
        ones_t = persist.tile([128, 1], F32)

        def dma_chunk(sc):
            t = xpool.tile([128, 8, QW], F32R, tag="xt", name=f"xt{sc}")
            nc.sync.dma_start(
                out=t,
                in_=io["xt"].ap()[:, QW * sc:QW * sc + QW].rearrange(
                    "(k p) s -> p k s", p=128
                ),
            )
            return t

        # DMA order: x chunk 0 first, then projection weights, small
        # constants, then the rest as needed.
        xt0 = dma_chunk(0)
        for w_s, name in ((wq_s, "wq"), (wkd_s, "wkd"), (wvg_s, "wvg")):
            nc.sync.dma_start(
                out=w_s, in_=io[name].ap().rearrange("(k p) m -> p k m", p=128)
            )
        nc.sync.dma_start(out=perm_s, in_=io["perm"][:, :])
        nc.sync.dma_start(out=mg_s, in_=io["mg"][:, :])
        nc.sync.dma_start(out=mones_s, in_=io["mones"][:, :])
        nc.sync.dma_start(out=msq_s, in_=io["msq"][:, :])
        nc.sync.dma_start(out=mok_s, in_=io["mok"][:, :])
        nc.sync.dma_start(out=ident_s, in_=io["ident"][:, :])
        nc.vector.memset(eps_t, EPS)
        nc.vector.memset(ones_t, 1.0)
        xt1 = dma_chunk(1)
        nc.sync.dma_start(out=c1_s, in_=io["c1"][:, :])
        nc.sync.dma_start(out=c2_s, in_=io["c2"][:, :])

        # ---------------- per-chunk projection + preprocessing -------------
        def prep(sc, xt_sc):
            sl = slice(QW * sc, QW * sc + QW)
            qt = scr.tile([128, QW], F32, tag="qt", name=f"qt{sc}")
            kt = scr.tile([128, QW], F32, tag="kt", name=f"kt{sc}")

            # q/k projections (k duplicated [k1;k2;k1;k2] by host weights)
            for w_s, dst, tag in ((wq_s, qt, "q"), (wkd_s, kt, "k")):
                acc = psOP.tile([128, QW], F32, tag="op", name=f"acc_{tag}{sc}")
                for kc in range(8):
                    nc.tensor.matmul(
                        acc, w_s[:, kc, :], xt_sc[:, kc, :],
                        start=(kc == 0), stop=(kc == 7),
                    )
                nc.vector.tensor_copy(_r(dst), acc)

            # sum-of-squares -> rms scales (natural_log_exp table only)
            qsq = scr.tile([128, QW], F32, tag="qsq", name=f"qsq{sc}")
            nc.vector.tensor_mul(_r(qsq), qt, qt)
            ksq = scr.tile([64, QW], F32, tag="ksq", name=f"ksq{sc}")
            nc.vector.tensor_mul(_r(ksq), kt[0:64, :], kt[0:64, :])
            sq = psOP.tile([2, QW], F32, tag="op", name=f"sq{sc}")
            nc.tensor.matmul(sq, msq_s, _r(qsq))
            sk = psOP.tile([1, QW], F32, tag="op", name=f"sk{sc}")
            nc.tensor.matmul(sk, mok_s, _r(ksq))
            # rsq = exp(-0.5 * ln(sumsq/64 + eps))
            lnq = scr.tile([2, QW], F32, tag="lnq", name=f"lnq{sc}")
            nc.scalar.activation(
                lnq, sq, LN, bias=eps_t[0:2, :], scale=1.0 / HD
            )
            rq = scr.tile([2, QW], F32, tag="rqk", name=f"rq{sc}")
            nc.scalar.activation(_r(rq), lnq, EXP, scale=-0.5)
            lnk = scr.tile([1, QW], F32, tag="lnk", name=f"lnk{sc}")
            nc.scalar.activation(
                lnk, sk, LN, bias=eps_t[0:1, :], scale=1.0 / HD
            )
            rk = scr.tile([1, QW], F32, tag="rk", name=f"rk{sc}")
            nc.scalar.activation(_r(rk), lnk, EXP, scale=-0.5)

            # broadcast scales over partitions (PE) + apply in place
            for mask, src_r, tgt in ((mg_s, rq, qt), (mones_s, rk, kt)):
                bc = psOP.tile([128, QW], F32, tag="op", name=f"bc{sc}")
                nc.tensor.matmul(bc, mask, _r(src_r))
                nc.vector.tensor_mul(_r(tgt), tgt, bc)

            # rotary (linear, post-scale): swap halves via PE perm matmul
            for src_t, dst in ((qt, qRc[sc]), (kt, kRc[sc])):
                sw = psOP.tile([128, QW], F32, tag="op", name=f"sw{sc}")
                nc.tensor.matmul(sw, perm_s, _r(src_t))
                t1 = scr.tile([128, QW], F32, tag="rot1", name=f"t1_{sc}")
                nc.vector.tensor_mul(t1, src_t, c1_s[:, sl])
                t2 = scr.tile([128, QW], F32, tag="rot2", name=f"t2_{sc}")
                nc.vector.tensor_mul(t2, sw, c2_s[:, sl])
                nc.vector.tensor_add(_r(dst[:, :]), t1, t2)

            # value projection + gate
            accv = psOP.tile([128, QW], F32, tag="op", name=f"accv{sc}")
            for kc in range(8):
                nc.tensor.matmul(
                    accv, wvg_s[:, kc, :], xt_sc[:, kc, :],
                    start=(kc == 0), stop=(kc == 7),
                )
            sg = scr.tile([64, QW], F32, tag="sg", name=f"sg{sc}")
            nc.scalar.activation(sg, accv[64:128, :], EXP, scale=-1.0)
            vt = scr.tile([64, QW], F32, tag="vt", name=f"vt{sc}")
            nc.vector.tensor_copy(vt, accv[0:64, :])
            nc.vector.tensor_scalar_add(sg, sg, 1.0)
            with nc.allow_low_precision(reason="sigmoid approx recip"):
                nc.vector.reciprocal_approx_fast(sg, sg)
            vga = scr.tile([64, QW], F32, tag="vga", name=f"vga{sc}")
            nc.vector.tensor_mul(vga, vt, sg)
            # transpose to natural layout [pos, dims]: 4 x 128-blocks into
            # one PSUM bank (start=True only on the first: the bank clear
            # of a matmul group covers the whole 2KB row of the written
            # partitions, later blocks overwrite their own columns), then
            # one strided copy into v_aug (ones columns interleaved so each
            # PV matmul also produces the softmax denominator).
            nc.vector.tensor_copy(
                _r(vac[sc][:, :, 32:33]), ones_t.to_broadcast((128, 4, 1))
            )
            nc.vector.tensor_copy(
                _r(vac[sc][:, :, 65:66]), ones_t.to_broadcast((128, 4, 1))
            )
            tva = psOP.tile([128, 4, HD], F32, tag="op", name=f"tva{sc}")
            for jj in range(4):
                nc.tensor.matmul(
                    tva[:, jj, :], vga[:, 128 * jj:128 * jj + 128], ident_s,
                    is_transpose=True, start=(jj == 0), stop=(jj == 3),
                    skip_group_check=True,
                )
            dst = vac[sc].rearrange("p a (b c) -> p a b c", b=2)[:, :, :, 0:32]
            src = tva.rearrange("p a (b c) -> p a b c", b=2)
            nc.vector.tensor_copy(_r(dst), src)

        # ---------------- attention super-block ----------------------------
        def attn(qs):
            nkb = 4 * qs + 4
            ut = utp.tile([128, QW], F32R, tag="ut", name=f"ut{qs}")
            for pr in range(2):
                at = [
                    psAT.tile([33, QW], F32, tag="at", name=f"at{qs}_{pr}_{j}")
                    for j in range(2)
                ]

                def pv(kb, pt):
                    off = max(0, 128 * (kb - 4 * qs))
                    cb, ib = kb // 4, kb % 4
                    for j in range(2):
                        su = 2 * pr + j
                        lo = 33 * (su % 2)
                        nc.tensor.matmul(
                            at[j][:, off:QW],
                            _r(vac[cb][:, ib, lo:lo + 33]),
                            _r(pt[:, j, off:QW]),
                            start=(kb == 0), stop=(kb == nkb - 1),
                            skip_group_check=True,
                        )

                prev = None
                for kb in range(nkb):
                    off = max(0, 128 * (kb - 4 * qs))
                    cb, ib = kb // 4, kb % 4
                    kcols = slice(128 * ib, 128 * ib + 128)
                    st = psST.tile(
                        [128, 2, QW], F32, tag="st", name=f"st{qs}_{kb}_{pr}"
                    )
                    for j in range(2):
                        su = 2 * pr + j
                        rows = slice(32 * su, 32 * su + 32)
                        nc.tensor.matmul(
                            st[:, j, off:QW],
                            _r(kRc[cb][rows, kcols]),
                            _r(qRc[qs][rows, off:QW]),
                            tile_position=(32 * su, 0),
                        )
                    pt = ptp.tile(
                        [128, 2, QW], F32, tag="pt", name=f"pt{qs}_{kb}_{pr}"
                    )
                    nc.scalar.activation(
                        _r(pt[:, :, off:QW]), st[:, :, off:QW], EXP
                    )
                    if kb >= 4 * qs:
                        # exact-diagonal block: zero strictly-upper (k>q)
                        for j in range(2):
                            nc.gpsimd.affine_select(
                                out=_r(pt[:, j, off:off + 128]),
                                in_=_r(pt[:, j, off:off + 128]),
                                compare_op=mybir.AluOpType.is_ge,
                                fill=0.0,
                                base=0,
                                pattern=[[1, 128]],
                                channel_multiplier=-1,
                            )
                    if DEBUG and qs == 0 and kb == 0:
                        nc.sync.dma_start(
                            out=io["dbg_pt"][:, 1024 * pr:1024 * (pr + 1)],
                            in_=pt.rearrange("p a b -> p (a b)"),
                        )
                    # software pipeline: PV for the previous block issues
                    # after this block's QK, so the PE fills the exp latency
                    if prev is not None:
                        pv(*prev)
                    prev = (kb, pt)
                pv(*prev)

                # normalize su pair: approx reciprocal of the denominator
                # row, K=1 PE broadcast over 32 partitions, multiply, then
                # SBUF->SBUF DMA into the su strip of uT.
                for j in range(2):
                    su = 2 * pr + j
                    atc = scr.tile([33, QW], F32, tag="atc", name=f"atc{qs}{su}")
                    nc.vector.tensor_copy(atc, at[j])
                    rcp = scr.tile([33, QW], F32, tag="rcp", name=f"rcp{qs}{su}")
                    with nc.allow_low_precision(reason="approx recip"):
                        nc.vector.reciprocal_approx_fast(rcp, atc)
                    rcr = scr.tile([33, QW], F32, tag="rcr", name=f"rcr{qs}{su}")
                    nc.vector.tensor_copy(_r(rcr[32:33, :]), rcp[32:33, :])
                    db = psOP.tile([32, QW], F32, tag="op", name=f"db{qs}{su}")
                    nc.tensor.matmul(
                        db,
                        onesb_s[32:33, :],
                        _r(rcr[32:33, :]),
                        tile_position=(32, 0),
                    )
                    utmp = scr.tile([32, QW], F32, tag="utmp", name=f"utm{qs}{su}")
                    nc.vector.tensor_mul(_r(utmp), atc[0:32, :], db)
                    nc.sync.dma_start(
                        out=ut[32 * su:32 * su + 32, :], in_=_r(utmp)
                    )
            if DEBUG:
                nc.sync.dma_start(
                    out=_r(io["dbg_ut"][:, QW * qs:QW * qs + QW]), in_=ut
                )

            # output projection for this q-super
            for sb in range(4):
                ob = obp.tile([128, DIM], F32, tag="ob", name=f"ob{qs}_{sb}")
                for ncn in range(2):
                    op = psOP.tile(
                        [128, QW], F32, tag="op", name=f"op{qs}_{sb}_{ncn}"
                    )
                    nc.tensor.matmul(
                        op,
                        ut[:, 128 * sb:128 * sb + 128],
                        wo_s[:, QW * ncn:QW * ncn + QW],
                    )
                    nc.vector.tensor_copy(ob[:, QW * ncn:QW * ncn + QW], op)
                row = QW * qs + 128 * sb
                nc.sync.dma_start(out=io["out"][row:row + 128, :], in_=ob)

        # ---------------- interleaved schedule ------------------------------
        prep(0, xt0)
        nc.sync.dma_start(out=wo_s, in_=io["wo"][:, :])
        nc.sync.dma_start(out=onesb_s, in_=io["onesb"][:, :])
        prep(1, xt1)
        xt2 = dma_chunk(2)
        attn(0)
        prep(2, xt2)
        xt3 = dma_chunk(3)
        attn(1)
        prep(3, xt3)
        attn(2)
        attn(3)

        if DEBUG:
            for i in range(QS):
                nc.sync.dma_start(
                    out=io["dbg_qR"][:, QW * i:QW * i + QW], in_=qRc[i]
                )
                nc.sync.dma_start(
                    out=io["dbg_kR"][:, QW * i:QW * i + QW], in_=kRc[i]
                )
                nc.sync.dma_start(
                    out=io["dbg_va"][:, 264 * i:264 * i + 264],
                    in_=vac[i].rearrange("p a b -> p (a b)"),
                )


@functools.lru_cache(maxsize=1)
def _build():
    nc = bacc.Bacc(
        "TRN2", target_bir_lowering=False, debug=False, num_devices=NCORES
    )
    io = {
        "xt": nc.dram_tensor("xt", [DIM, S], F32R, kind="ExternalInput"),
        "wq": nc.dram_tensor("wq", [DIM, 128], F32R, kind="ExternalInput"),
        "wkd": nc.dram_tensor("wkd", [DIM, 128], F32R, kind="ExternalInput"),
        "wvg": nc.dram_tensor("wvg", [DIM, 128], F32R, kind="ExternalInput"),
        "wo": nc.dram_tensor("wo", [128, DIM], F32R, kind="ExternalInput"),
        "c1": nc.dram_tensor("c1", [128, S], F32, kind="ExternalInput"),
        "c2": nc.dram_tensor("c2", [128, S], F32, kind="ExternalInput"),
        "perm": nc.dram_tensor("perm", [128, 128], F32R, kind="ExternalInput"),
        "ident": nc.dram_tensor("ident", [64, 64], F32, kind="ExternalInput"),
        "mg": nc.dram_tensor("mg", [2, 128], F32R, kind="ExternalInput"),
        "mones": nc.dram_tensor("mones", [1, 128], F32R, kind="ExternalInput"),
        "msq": nc.dram_tensor("msq", [128, 2], F32R, kind="ExternalInput"),
        "mok": nc.dram_tensor("mok", [64, 1], F32R, kind="ExternalInput"),
        "onesb": nc.dram_tensor("onesb", [33, 32], F32R, kind="ExternalInput"),
        "out": nc.dram_tensor("out", [S, DIM], F32, kind="ExternalOutput"),
    }
    if DEBUG:
        io["dbg_qR"] = nc.dram_tensor("dbg_qR", [128, S], F32, kind="ExternalOutput")
        io["dbg_kR"] = nc.dram_tensor("dbg_kR", [128, S], F32, kind="ExternalOutput")
        io["dbg_va"] = nc.dram_tensor(
            "dbg_va", [128, 16 * 66], F32, kind="ExternalOutput"
        )
        io["dbg_pt"] = nc.dram_tensor(
            "dbg_pt", [128, 4 * 512], F32, kind="ExternalOutput"
        )
        io["dbg_ut"] = nc.dram_tensor("dbg_ut", [128, S], F32, kind="ExternalOutput")
    with tile.TileContext(nc) as tc:
        _build_kernel(tc, io)
    nc.compile()
    return nc


def _tf32(x):
    """Round f32 array to tfloat32 bit pattern (RNE-ish) so the PE's f32r
    truncation is exact on pre-rounded data."""
    b = np.ascontiguousarray(x, np.float32).view(np.uint32)
    out = ((b + np.uint32(0x00001000)) & np.uint32(0xFFFFE000)).view(np.float32)
    return np.ascontiguousarray(out)


def _host_tables():
    i = np.arange(0, HD, 2, dtype=np.float32) / HD * math.pi  # [32]
    pos = np.arange(S, dtype=np.float32)
    radius = 1.0 / (1.0 + pos[:, None] * 0.01)
    ang = pos[:, None] * i[None, :]
    cosT = np.ascontiguousarray((radius * np.cos(ang)).T.astype(np.float32))
    sinT = np.ascontiguousarray((radius * np.sin(ang)).T.astype(np.float32))
    c1 = np.tile(cosT, (4, 1))
    c2 = np.concatenate([sinT, -sinT, sinT, -sinT], 0)
    perm = np.zeros((128, 128), np.float32)
    for m in range(128):
        perm[(m // 64) * 64 + ((m + 32) % 64), m] = 1.0
    msq = np.zeros((128, 2), np.float32)
    msq[0:64, 0] = 1.0
    msq[64:128, 1] = 1.0
    return c1, c2, perm, msq


def make_in_maps(inputs):
    x = np.asarray(inputs["x"], np.float32)
    Wq = np.asarray(inputs["Wq"], np.float32)
    Wk = np.asarray(inputs["Wk"], np.float32)
    Wv = np.asarray(inputs["Wv"], np.float32)
    Wg = np.asarray(inputs["Wg"], np.float32)
    Wo = np.asarray(inputs["Wo"], np.float32)
    q_gain = np.asarray(inputs["q_gain"], np.float32)
    lam = np.asarray(inputs["lambda_param"], np.float32)

    xT = np.ascontiguousarray(x[0].T)  # [DIM, S]
    c1, c2, perm, msq = _host_tables()
    ident = np.eye(64, dtype=np.float32)
    mok = np.ones((64, 1), np.float32)
    onesb = np.zeros((33, 32), np.float32)
    onesb[32] = 1.0
    mones = np.ones((1, 128), np.float32)

    in_maps = []
    for c in range(NCORES):
        g = c // 2
        h0, h1 = 2 * c, 2 * c + 1
        Wk_g = Wk[64 * g:64 * g + 64]
        Wv_g = Wv[64 * g:64 * g + 64]
        Wg_g = Wg[64 * g:64 * g + 64]
        mg = np.zeros((2, 128), np.float32)
        mg[0, 0:64] = q_gain[h0] / math.sqrt(HALF)
        mg[1, 64:128] = q_gain[h1] / math.sqrt(HALF)
        woP = np.zeros((128, DIM), np.float32)
        for i, h in enumerate((h0, h1)):
            W1 = Wo[:, 64 * h:64 * h + 32]
            W2 = Wo[:, 64 * h + 32:64 * h + 64]
            woP[64 * i:64 * i + 32] = (W1 + W2).T
            woP[64 * i + 32:64 * i + 64] = (lam[h] * (W2 - W1)).T
        in_maps.append({
            "xt": _tf32(xT),
            "wq": _tf32(Wq[128 * c:128 * c + 128].T),
            "wkd": _tf32(np.concatenate([Wk_g, Wk_g], 0).T),
            "wvg": _tf32(np.concatenate([Wv_g, Wg_g], 0).T),
            "wo": _tf32(woP),
            "c1": c1,
            "c2": c2,
            "perm": perm,
            "ident": ident,
            "mg": _tf32(mg),
            "mones": mones,
            "msq": msq,
            "mok": mok,
            "onesb": onesb,
        })
    return in_maps


def kernel(**inputs):
    nc = _build()
    in_maps = make_in_maps(inputs)
    res = run_bass_kernel_spmd(nc, in_maps, core_ids=list(range(NCORES)))
    total = np.zeros((S, DIM), np.float32)
    for c in range(NCORES):
        total += res.results[c]["out"]
    return total.reshape(1, S, DIM)
